# revision 14
# baseline (speedup 1.0000x reference)
"""NCC loss (VoxelMorph-style, 9^3 box window) on 8 Trainium2 NeuronCores.

Data-parallel over depth: each core handles 16 output slices (+4-slice halos)
for both batch elements.  Per core, for each of 5 volumes (I, J, I*J, I^2, J^2):

  win3-D (stride-3):  s[d] = x[d] + x[d+3] + x[d+6]          (DVE, bf16 2x)
  pass A:  per s-slice matmul, lhsT = data (stationary), rhs = banded-ones BB
           -> box-sums the H axis AND transposes to [W, H'] in PSUM   (PE)
  drain:   PSUM -> SBUF bf16 y tiles                          (ACT copies)
  pass B:  lhsT = BB (stationary), rhs = y streamed 512 wide; 3 accumulating
           MMs with d-offsets 0,1,2 complete the win9-D sum while box-summing
           W -> full box sums in PSUM                                 (PE)
  NCC per 4-slice group (direct from the 5 PSUM banks):
           si = SI/27, sj = SJ/27 (ACT copies); products si^2 etc (DVE 2x);
           A' = I2s - si^2, B' = J2s - sj^2, C' = IJs - si*sj  (DVE, 1x PSUM)
           cc = C' * exp(-0.5*ln(A'*B'))  (ACT Ln/Exp), fused group reduce.

Head optimizations: batch-0 inputs stream chunked over two parallel DMA rings
(I on the sync HWDGE ring, J on the gpsimd SWDGE ring) and the first win3s
are split so compute starts as soon as the first half-volume lands; squares
are interleaved between per-volume drain blocks on ACT so the early drains
never sit behind them.  Host sums the 8 x [128 x 8] partials: 1 - total/N.
"""

from contextlib import ExitStack

import numpy as np

WIN = 9
PAD = WIN // 2  # 4
B = 2
D = 128
H = 128
W = 128
NCORES = 8
D_OUT = D // NCORES  # 16
D_IN = D_OUT + 2 * PAD  # 24
NS = D_IN - 6  # 18 stride-3 win3 slices
WIN_SIZE = 729.0
N_TOTAL = float(B * D * H * W)
NG = 2  # 8-slice group-pairs per batch (each spans 2 PSUM banks)
GS = D_OUT // NG  # 8 slices per group

_CACHE = {}


def _split_multiwaits(nc):
    """Walrus in this env encodes at most ONE sync-wait per instruction.
    Hoist extra waits onto standalone EventSemaphore insts just before."""
    from concourse import mybir

    n = 0
    for fn in nc.m.functions:
        for bb in fn.blocks:
            il = bb.instructions
            out = []
            for inst in il:
                si = inst.sync_info
                if si is not None and si.on_wait and len(si.on_wait) > 1:
                    waits = list(si.on_wait)
                    for w in waits[:-1]:
                        ev = mybir.InstEventSemaphore(
                            name=f"EVW-{n}", ins=[], outs=[])
                        n += 1
                        ev.engine = inst.engine
                        ev.sync_info = mybir.SyncInfo(on_wait=[w],
                                                      on_update=[])
                        out.append(ev)
                    inst.sync_info = mybir.SyncInfo(
                        on_wait=[waits[-1]], on_update=list(si.on_update))
                out.append(inst)
            il[:] = out
    return n


VOLS = ("I", "J", "IJ", "I2", "J2")
CHUNKS = ((0, 4), (4, 4), (8, 4), (12, 4), (16, 2))  # pass-A psum chunks
HC = 12  # DMA half-chunk (slices) for batch 0


def _build_nc():
    import concourse.bass as bass
    import concourse.tile as tile
    from concourse import mybir

    f32 = mybir.dt.float32
    bf16 = mybir.dt.bfloat16
    Alu = mybir.AluOpType
    Act = mybir.ActivationFunctionType

    nc = bass.Bass()
    I_ext = nc.declare_dram_parameter("I", [B, H, D_IN * W], bf16,
                                      isOutput=False)
    J_ext = nc.declare_dram_parameter("J", [B, H, D_IN * W], bf16,
                                      isOutput=False)
    BB_ext = nc.declare_dram_parameter("BB", [H, H], bf16, isOutput=False)
    out_ext = nc.declare_dram_parameter("partials", [128, B * NG], f32,
                                        isOutput=True)

    with tile.TileContext(nc) as tc, ExitStack() as ctx:
        const = ctx.enter_context(tc.tile_pool(name="const", bufs=1))
        src = ctx.enter_context(tc.tile_pool(name="src", bufs=2))
        prod = ctx.enter_context(tc.tile_pool(name="prod", bufs=2))
        wtmp = ctx.enter_context(tc.tile_pool(name="wtmp", bufs=2))
        sp = ctx.enter_context(tc.tile_pool(name="sp", bufs=4))
        yp = ctx.enter_context(tc.tile_pool(name="yp", bufs=6))
        nccb = ctx.enter_context(tc.tile_pool(name="nccb", bufs=2))
        pout = ctx.enter_context(tc.tile_pool(name="pout", bufs=1))
        pa = ctx.enter_context(tc.tile_pool(name="pa", bufs=2, space="PSUM"))
        pb = ctx.enter_context(tc.tile_pool(name="pb", bufs=3, space="PSUM"))

        BBt = const.tile([H, H], bf16)
        nc.sync.dma_start(out=BBt, in_=BB_ext[:, :])
        partsT = pout.tile([128, B * NG], f32)

        # inputs: I volumes on the sync HWDGE ring, J volumes on the gpsimd
        # SWDGE ring -> the two streams run in parallel.  Batch 0 is split
        # in half so compute can start on the first 12 slices.
        hc = HC * W
        tI, tJ = [], []
        for b in range(B):
            ti = src.tile([H, D_IN * W], bf16, tag="tI")
            tj = src.tile([H, D_IN * W], bf16, tag="tJ")
            if b == 0:
                nc.sync.dma_start(out=ti[:, :hc], in_=I_ext[b][:, :hc])
                nc.sync.dma_start(out=tj[:, :hc], in_=J_ext[b][:, :hc])
                nc.sync.dma_start(out=ti[:, hc:], in_=I_ext[b][:, hc:])
                nc.sync.dma_start(out=tj[:, hc:], in_=J_ext[b][:, hc:])
            else:
                nc.sync.dma_start(out=ti, in_=I_ext[b])
                nc.sync.dma_start(out=tj, in_=J_ext[b])
            tI.append(ti)
            tJ.append(tj)

        n3 = NS * W

        def win3(x):
            a = wtmp.tile([H, NS * W], bf16, tag="a")
            s = sp.tile([H, NS * W], bf16, tag="s")
            nc.vector.tensor_add(out=a, in0=x[:, 0:n3],
                                 in1=x[:, 3 * W:3 * W + n3])
            nc.vector.tensor_add(out=s, in0=a, in1=x[:, 6 * W:6 * W + n3])
            return s

        def win3_chunked(x):
            # split at slice 6 so part 1 only needs input slices [0, 12)
            a = wtmp.tile([H, NS * W], bf16, tag="a")
            s = sp.tile([H, NS * W], bf16, tag="s")
            c = 6 * W
            nc.vector.tensor_add(out=a[:, :c], in0=x[:, 0:c],
                                 in1=x[:, 3 * W:3 * W + c])
            nc.vector.tensor_add(out=s[:, :c], in0=a[:, :c],
                                 in1=x[:, 6 * W:6 * W + c])
            n2 = n3 - c
            nc.vector.tensor_add(out=a[:, c:], in0=x[:, c:c + n2],
                                 in1=x[:, c + 3 * W:c + 3 * W + n2])
            nc.vector.tensor_add(out=s[:, c:], in0=a[:, c:],
                                 in1=x[:, c + 6 * W:c + 6 * W + n2])
            return s

        yvols = [{} for _ in range(B)]

        def pass_a_vol(b, v, s):
            y = yp.tile([H, NS * W], bf16, tag="y")
            yvols[b][v] = y
            for k0, kn in CHUNKS:
                pat = pa.tile([128, 512], f32, tag="pa")
                for j in range(kn):
                    k = k0 + j
                    nc.tensor.matmul(out=pat[:, j * 128:(j + 1) * 128],
                                     lhsT=s[:, k * W:(k + 1) * W],
                                     rhs=BBt, start=True, stop=True)
                nc.scalar.copy(out=y[:, k0 * W:(k0 + kn) * W],
                               in_=pat[:, :kn * 128])

        # ---- batch 0 head: interleave squares / win3 / pass A per volume
        ti2_0 = prod.tile([H, D_IN * W], bf16, tag="tI2")
        tj2_0 = prod.tile([H, D_IN * W], bf16, tag="tJ2")
        tij_0 = prod.tile([H, D_IN * W], bf16, tag="tIJ")
        ti2_1 = prod.tile([H, D_IN * W], bf16, tag="tI2")
        tj2_1 = prod.tile([H, D_IN * W], bf16, tag="tJ2")
        tij_1 = prod.tile([H, D_IN * W], bf16, tag="tIJ")

        s_bI = win3_chunked(tI[0])
        nc.scalar.activation(out=ti2_0, in_=tI[0], func=Act.Square)
        pass_a_vol(0, "I", s_bI)
        s_bJ = win3_chunked(tJ[0])
        nc.scalar.activation(out=tj2_0, in_=tJ[0], func=Act.Square)
        pass_a_vol(0, "J", s_bJ)
        nc.vector.tensor_mul(out=tij_0, in0=tI[0], in1=tJ[0])
        s_bIJ = win3(tij_0)
        nc.scalar.activation(out=ti2_1, in_=tI[1], func=Act.Square)
        pass_a_vol(0, "IJ", s_bIJ)
        s_bI2 = win3(ti2_0)
        nc.scalar.activation(out=tj2_1, in_=tJ[1], func=Act.Square)
        pass_a_vol(0, "I2", s_bI2)
        s_bJ2 = win3(tj2_0)
        pass_a_vol(0, "J2", s_bJ2)

        # ---- batch 1 win3 (DVE) while batch-0 drains / pass B run
        s1 = {}
        s1["I"] = win3(tI[1])
        s1["J"] = win3(tJ[1])
        nc.vector.tensor_mul(out=tij_1, in0=tI[1], in1=tJ[1])
        s1["IJ"] = win3(tij_1)
        s1["I2"] = win3(ti2_1)
        s1["J2"] = win3(tj2_1)

        NW = GS * W  # 1024: NCC op width (2 PSUM banks)

        def pass_b_vol(b, g, v):
            # 8-slice group: two 4-slice accumulation chains, one per bank
            pt = pb.tile([128, NW], f32, tag="pb")
            y = yvols[b][v]
            for h in range(2):
                d0 = g * GS + h * 4
                for m in range(3):
                    nc.tensor.matmul(
                        out=pt[:, h * 512:(h + 1) * 512], lhsT=BBt,
                        rhs=y[:, (d0 + m) * W:(d0 + m + 4) * W],
                        start=(m == 0), stop=(m == 2))
            return pt

        def pass_b_pre(b, g):
            # wave order keeps peak PSUM at 3x2 banks: I,J -> cp frees them
            # for J2,IJ via pool-slot reuse
            t = {}
            t["I"] = pass_b_vol(b, g, "I")
            t["J"] = pass_b_vol(b, g, "J")
            si = nccb.tile([128, NW], bf16, tag="si")
            sj = nccb.tile([128, NW], bf16, tag="sj")
            nc.scalar.activation(out=si, in_=t["I"], func=Act.Copy,
                                 scale=1.0 / 27.0)
            nc.scalar.activation(out=sj, in_=t["J"], func=Act.Copy,
                                 scale=1.0 / 27.0)
            t["I2"] = pass_b_vol(b, g, "I2")
            t["J2"] = pass_b_vol(b, g, "J2")
            t["IJ"] = pass_b_vol(b, g, "IJ")
            sa = nccb.tile([128, NW], bf16, tag="sa")
            sb = nccb.tile([128, NW], bf16, tag="sb")
            sc = nccb.tile([128, NW], bf16, tag="sc")
            nc.vector.tensor_mul(out=sa, in0=si, in1=si)
            nc.vector.tensor_mul(out=sb, in0=sj, in1=sj)
            nc.vector.tensor_mul(out=sc, in0=si, in1=sj)
            t["sa"], t["sb"], t["sc"] = sa, sb, sc
            return t

        def ncc_mid(t):
            # /729 domain: A' = I2s - (SI/27)^2 = I_var, etc.
            va = nccb.tile([128, NW], bf16, tag="va")
            vb = nccb.tile([128, NW], bf16, tag="vb")
            vc = nccb.tile([128, NW], bf16, tag="vc")
            p = nccb.tile([128, NW], bf16, tag="p")
            nc.vector.tensor_sub(out=va, in0=t["I2"], in1=t["sa"])
            nc.vector.tensor_sub(out=vb, in0=t["J2"], in1=t["sb"])
            nc.vector.tensor_sub(out=vc, in0=t["IJ"], in1=t["sc"])
            nc.vector.tensor_mul(out=p, in0=va, in1=vb)
            t["vc"], t["p"] = vc, p

        def ncc_act(t):
            q = nccb.tile([128, NW], bf16, tag="q")
            r = nccb.tile([128, NW], bf16, tag="r")
            nc.scalar.activation(out=q, in_=t["p"], func=Act.Ln)
            nc.scalar.activation(out=r, in_=q, func=Act.Exp, scale=-0.5)
            t["r"] = r

        def ncc_fin(t, gi):
            fin = nccb.tile([128, NW], bf16, tag="fin")
            nc.vector.scalar_tensor_tensor(
                out=fin, in0=t["vc"], scalar=0.0, in1=t["r"],
                op0=Alu.add, op1=Alu.mult,
                accum_out=partsT[:, gi:gi + 1])

        # ---- batch 0 groups: pass B + NCC-pre
        t0 = []
        for g in range(NG):
            t0.append(pass_b_pre(0, g))

        # ---- batch 1 pass A + drains (ACT) while batch-0 NCC main runs
        for v in VOLS:
            pass_a_vol(1, v, s1[v])

        # ---- batch 0 NCC main
        for t in t0:
            ncc_mid(t)
        for t in t0:
            ncc_act(t)
        for g, t in enumerate(t0):
            ncc_fin(t, g)

        # ---- batch 1 groups
        t1 = []
        for g in range(NG):
            t1.append(pass_b_pre(1, g))
        for t in t1:
            ncc_mid(t)
        for t in t1:
            ncc_act(t)
        for g, t in enumerate(t1):
            ncc_fin(t, NG + g)

        nc.sync.dma_start(out=out_ext[:, :], in_=partsT)

    return nc


def _get_nc(split=True):
    if "nc" not in _CACHE:
        _CACHE["nc"] = _build_nc()
    if split and not _CACHE.get("split"):
        _split_multiwaits(_CACHE["nc"])
        _CACHE["split"] = True
    return _CACHE["nc"]


def _shards(y_true, y_pred):
    import ml_dtypes

    yt = np.ascontiguousarray(
        np.asarray(y_true, dtype=np.float32).reshape(B, D, H, W))
    yp = np.ascontiguousarray(
        np.asarray(y_pred, dtype=np.float32).reshape(B, D, H, W))
    pt = np.zeros((B, D + 2 * PAD, H, W), dtype=ml_dtypes.bfloat16)
    pp = np.zeros((B, D + 2 * PAD, H, W), dtype=ml_dtypes.bfloat16)
    pt[:, PAD:PAD + D] = yt.astype(ml_dtypes.bfloat16)
    pp[:, PAD:PAD + D] = yp.astype(ml_dtypes.bfloat16)

    BB = np.zeros((H, H), dtype=np.float32)
    for i in range(H):
        BB[i, max(0, i - PAD):min(H, i + PAD + 1)] = 1.0
    BB_bf16 = BB.astype(ml_dtypes.bfloat16)

    in_maps = []
    for c in range(NCORES):
        lo = c * D_OUT
        # transpose to [B, H, D_IN, W] so the on-device layout (partition=H)
        # is a fully contiguous DMA
        icore = np.ascontiguousarray(
            pt[:, lo:lo + D_IN].transpose(0, 2, 1, 3)).reshape(B, H, -1)
        jcore = np.ascontiguousarray(
            pp[:, lo:lo + D_IN].transpose(0, 2, 1, 3)).reshape(B, H, -1)
        in_maps.append({"I": icore, "J": jcore, "BB": BB_bf16})
    return in_maps


def run(y_true, y_pred, trace=False):
    from concourse.bass_utils import run_bass_kernel_spmd

    nc = _get_nc()
    in_maps = _shards(y_true, y_pred)
    res = run_bass_kernel_spmd(nc, in_maps, list(range(NCORES)), trace=trace)
    total = 0.0
    for r in res.results:
        total += float(np.asarray(r["partials"], dtype=np.float64).sum())
    loss = np.float32(1.0 - total / N_TOTAL)
    return np.array(loss, dtype=np.float32), res


def kernel(y_true, y_pred):
    loss, _ = run(y_true, y_pred, trace=False)
    return loss


# revision 17
# speedup vs baseline: 1.1525x; 1.1525x over previous
"""NCC loss (VoxelMorph-style, 9^3 box window) on 8 Trainium2 NeuronCores.

Data-parallel over depth: each core handles 16 output slices (+4-slice halos)
for both batch elements.  Per core, for each of 5 volumes (I, J, I*J, I^2, J^2):

  win3-D (stride-3):  s[d] = x[d] + x[d+3] + x[d+6]          (DVE, bf16 2x)
  pass A:  per s-slice matmul, lhsT = data (stationary), rhs = banded-ones BB
           -> box-sums the H axis AND transposes to [W, H'] in PSUM   (PE)
  drain:   PSUM -> SBUF bf16 y tiles                          (ACT copies)
  pass B:  lhsT = BB (stationary), rhs = y streamed 512 wide; 3 accumulating
           MMs with d-offsets 0,1,2 complete the win9-D sum while box-summing
           W -> full box sums in PSUM                                 (PE)
  NCC per 4-slice group (direct from the 5 PSUM banks):
           si = SI/27, sj = SJ/27 (ACT copies); products si^2 etc (DVE 2x);
           A' = I2s - si^2, B' = J2s - sj^2, C' = IJs - si*sj  (DVE, 1x PSUM)
           cc = C' * exp(-0.5*ln(A'*B'))  (ACT Ln/Exp), fused group reduce.

Head optimizations: batch-0 inputs stream chunked over two parallel DMA rings
(I on the sync HWDGE ring, J on the gpsimd SWDGE ring) and the first win3s
are split so compute starts as soon as the first half-volume lands; squares
are interleaved between per-volume drain blocks on ACT so the early drains
never sit behind them.  Host sums the 8 x [128 x 8] partials: 1 - total/N.
"""

from contextlib import ExitStack

import numpy as np

WIN = 9
PAD = WIN // 2  # 4
B = 2
D = 128
H = 128
W = 128
NCORES = 8
D_OUT = D // NCORES  # 16
D_IN = D_OUT + 2 * PAD  # 24
NS = D_IN - 6  # 18 stride-3 win3 slices
WIN_SIZE = 729.0
N_TOTAL = float(B * D * H * W)
NG = 2  # 8-slice group-pairs per batch (each spans 2 PSUM banks)
GS = D_OUT // NG  # 8 slices per group

_CACHE = {}


def _split_multiwaits(nc):
    """Walrus in this env encodes at most ONE sync-wait per instruction.
    Hoist extra waits onto standalone EventSemaphore insts just before."""
    from concourse import mybir

    n = 0
    for fn in nc.m.functions:
        for bb in fn.blocks:
            il = bb.instructions
            out = []
            for inst in il:
                si = inst.sync_info
                if si is not None and si.on_wait and len(si.on_wait) > 1:
                    waits = list(si.on_wait)
                    for w in waits[:-1]:
                        ev = mybir.InstEventSemaphore(
                            name=f"EVW-{n}", ins=[], outs=[])
                        n += 1
                        ev.engine = inst.engine
                        ev.sync_info = mybir.SyncInfo(on_wait=[w],
                                                      on_update=[])
                        out.append(ev)
                    inst.sync_info = mybir.SyncInfo(
                        on_wait=[waits[-1]], on_update=list(si.on_update))
                out.append(inst)
            il[:] = out
    return n


VOLS = ("I", "J", "IJ", "I2", "J2")
CHUNKS = ((0, 4), (4, 4), (8, 4), (12, 4), (16, 2))  # pass-A psum chunks
HC = 12  # DMA half-chunk (slices) for batch 0


def _build_nc():
    import concourse.bass as bass
    import concourse.tile as tile
    from concourse import mybir

    f32 = mybir.dt.float32
    bf16 = mybir.dt.bfloat16
    Alu = mybir.AluOpType
    Act = mybir.ActivationFunctionType

    nc = bass.Bass()
    I_ext = nc.declare_dram_parameter("I", [B, H, D_IN * W], bf16,
                                      isOutput=False)
    J_ext = nc.declare_dram_parameter("J", [B, H, D_IN * W], bf16,
                                      isOutput=False)
    BB_ext = nc.declare_dram_parameter("BB", [H, H], bf16, isOutput=False)
    out_ext = nc.declare_dram_parameter("partials", [128, B * NG], f32,
                                        isOutput=True)

    with tile.TileContext(nc) as tc, ExitStack() as ctx:
        const = ctx.enter_context(tc.tile_pool(name="const", bufs=1))
        src = ctx.enter_context(tc.tile_pool(name="src", bufs=2))
        prod = ctx.enter_context(tc.tile_pool(name="prod", bufs=2))
        wtmp = ctx.enter_context(tc.tile_pool(name="wtmp", bufs=2))
        sp = ctx.enter_context(tc.tile_pool(name="sp", bufs=4))
        yp = ctx.enter_context(tc.tile_pool(name="yp", bufs=6))
        nccb = ctx.enter_context(tc.tile_pool(name="nccb", bufs=2))
        pout = ctx.enter_context(tc.tile_pool(name="pout", bufs=1))
        pa = ctx.enter_context(tc.tile_pool(name="pa", bufs=2, space="PSUM"))
        pb = ctx.enter_context(tc.tile_pool(name="pb", bufs=3, space="PSUM"))

        BBt = const.tile([H, H], bf16)
        partsT = pout.tile([128, B * NG], f32)

        # inputs on the sync HWDGE ring.  Batch 0 is split in half so compute
        # can start on the first 12 slices; BB (small packets) is issued
        # after the batch-0 chunks so it doesn't delay them.
        hc = HC * W
        tI, tJ = [], []
        for b in range(B):
            ti = src.tile([H, D_IN * W], bf16, tag="tI")
            tj = src.tile([H, D_IN * W], bf16, tag="tJ")
            if b == 0:
                nc.sync.dma_start(out=ti[:, :hc], in_=I_ext[b][:, :hc])
                nc.sync.dma_start(out=tj[:, :hc], in_=J_ext[b][:, :hc])
                nc.sync.dma_start(out=ti[:, hc:], in_=I_ext[b][:, hc:])
                nc.sync.dma_start(out=tj[:, hc:], in_=J_ext[b][:, hc:])
                nc.sync.dma_start(out=BBt, in_=BB_ext[:, :])
            else:
                nc.sync.dma_start(out=ti, in_=I_ext[b])
                nc.sync.dma_start(out=tj, in_=J_ext[b])
            tI.append(ti)
            tJ.append(tj)

        n3 = NS * W

        def win3(x):
            a = wtmp.tile([H, NS * W], bf16, tag="a")
            s = sp.tile([H, NS * W], bf16, tag="s")
            nc.vector.tensor_add(out=a, in0=x[:, 0:n3],
                                 in1=x[:, 3 * W:3 * W + n3])
            nc.vector.tensor_add(out=s, in0=a, in1=x[:, 6 * W:6 * W + n3])
            return s

        def win3_chunked(x):
            # split at slice 6 so part 1 only needs input slices [0, 12)
            a = wtmp.tile([H, NS * W], bf16, tag="a")
            s = sp.tile([H, NS * W], bf16, tag="s")
            c = 6 * W
            nc.vector.tensor_add(out=a[:, :c], in0=x[:, 0:c],
                                 in1=x[:, 3 * W:3 * W + c])
            nc.vector.tensor_add(out=s[:, :c], in0=a[:, :c],
                                 in1=x[:, 6 * W:6 * W + c])
            n2 = n3 - c
            nc.vector.tensor_add(out=a[:, c:], in0=x[:, c:c + n2],
                                 in1=x[:, c + 3 * W:c + 3 * W + n2])
            nc.vector.tensor_add(out=s[:, c:], in0=a[:, c:],
                                 in1=x[:, c + 6 * W:c + 6 * W + n2])
            return s

        yvols = [{} for _ in range(B)]

        def pass_a_vol(b, v, s, split_drains=False):
            # split_drains: alternate PSUM drains between DVE and ACT so the
            # drain phase halves in wall-clock (used for batch 1, where the
            # DVE would otherwise idle waiting on ACT)
            y = yp.tile([H, NS * W], bf16, tag="y")
            yvols[b][v] = y
            for ci, (k0, kn) in enumerate(CHUNKS):
                pat = pa.tile([128, 512], f32, tag="pa")
                for j in range(kn):
                    k = k0 + j
                    nc.tensor.matmul(out=pat[:, j * 128:(j + 1) * 128],
                                     lhsT=s[:, k * W:(k + 1) * W],
                                     rhs=BBt, start=True, stop=True)
                if split_drains and ci in (0, 2):
                    nc.vector.tensor_copy(out=y[:, k0 * W:(k0 + kn) * W],
                                          in_=pat[:, :kn * 128])
                else:
                    nc.scalar.copy(out=y[:, k0 * W:(k0 + kn) * W],
                                   in_=pat[:, :kn * 128])

        # ---- batch 0 head: interleave squares / win3 / pass A per volume
        ti2_0 = prod.tile([H, D_IN * W], bf16, tag="tI2")
        tj2_0 = prod.tile([H, D_IN * W], bf16, tag="tJ2")
        tij_0 = prod.tile([H, D_IN * W], bf16, tag="tIJ")
        ti2_1 = prod.tile([H, D_IN * W], bf16, tag="tI2")
        tj2_1 = prod.tile([H, D_IN * W], bf16, tag="tJ2")
        tij_1 = prod.tile([H, D_IN * W], bf16, tag="tIJ")

        s_bI = win3_chunked(tI[0])
        nc.scalar.activation(out=ti2_0, in_=tI[0], func=Act.Square)
        pass_a_vol(0, "I", s_bI)
        s_bJ = win3_chunked(tJ[0])
        nc.scalar.activation(out=tj2_0, in_=tJ[0], func=Act.Square)
        pass_a_vol(0, "J", s_bJ)
        nc.vector.tensor_mul(out=tij_0, in0=tI[0], in1=tJ[0])
        s_bIJ = win3(tij_0)
        nc.scalar.activation(out=ti2_1, in_=tI[1], func=Act.Square)
        pass_a_vol(0, "IJ", s_bIJ)
        s_bI2 = win3(ti2_0)
        nc.scalar.activation(out=tj2_1, in_=tJ[1], func=Act.Square)
        pass_a_vol(0, "I2", s_bI2)
        s_bJ2 = win3(tj2_0)
        pass_a_vol(0, "J2", s_bJ2)

        # ---- batch 1 win3 (DVE) while batch-0 drains / pass B run
        s1 = {}
        s1["I"] = win3(tI[1])
        s1["J"] = win3(tJ[1])
        nc.vector.tensor_mul(out=tij_1, in0=tI[1], in1=tJ[1])
        s1["IJ"] = win3(tij_1)
        s1["I2"] = win3(ti2_1)
        s1["J2"] = win3(tj2_1)

        NW = GS * W  # 1024: NCC op width (2 PSUM banks)

        def pass_b_vol(b, g, v):
            # 8-slice group: two 4-slice accumulation chains, one per bank
            pt = pb.tile([128, NW], f32, tag="pb")
            y = yvols[b][v]
            for h in range(2):
                d0 = g * GS + h * 4
                for m in range(3):
                    nc.tensor.matmul(
                        out=pt[:, h * 512:(h + 1) * 512], lhsT=BBt,
                        rhs=y[:, (d0 + m) * W:(d0 + m + 4) * W],
                        start=(m == 0), stop=(m == 2))
            return pt

        def pass_b_pre(b, g):
            # wave order keeps peak PSUM at 3x2 banks: I,J -> cp frees them
            # for J2,IJ via pool-slot reuse
            t = {}
            t["I"] = pass_b_vol(b, g, "I")
            t["J"] = pass_b_vol(b, g, "J")
            si = nccb.tile([128, NW], bf16, tag="si")
            sj = nccb.tile([128, NW], bf16, tag="sj")
            nc.scalar.activation(out=si, in_=t["I"], func=Act.Copy,
                                 scale=1.0 / 27.0)
            nc.scalar.activation(out=sj, in_=t["J"], func=Act.Copy,
                                 scale=1.0 / 27.0)
            t["I2"] = pass_b_vol(b, g, "I2")
            t["J2"] = pass_b_vol(b, g, "J2")
            t["IJ"] = pass_b_vol(b, g, "IJ")
            sa = nccb.tile([128, NW], bf16, tag="sa")
            sb = nccb.tile([128, NW], bf16, tag="sb")
            sc = nccb.tile([128, NW], bf16, tag="sc")
            nc.vector.tensor_mul(out=sa, in0=si, in1=si)
            nc.vector.tensor_mul(out=sb, in0=sj, in1=sj)
            nc.vector.tensor_mul(out=sc, in0=si, in1=sj)
            t["sa"], t["sb"], t["sc"] = sa, sb, sc
            return t

        def ncc_mid(t):
            # /729 domain: A' = I2s - (SI/27)^2 = I_var, etc.
            va = nccb.tile([128, NW], bf16, tag="va")
            vb = nccb.tile([128, NW], bf16, tag="vb")
            vc = nccb.tile([128, NW], bf16, tag="vc")
            p = nccb.tile([128, NW], bf16, tag="p")
            nc.vector.tensor_sub(out=va, in0=t["I2"], in1=t["sa"])
            nc.vector.tensor_sub(out=vb, in0=t["J2"], in1=t["sb"])
            nc.vector.tensor_sub(out=vc, in0=t["IJ"], in1=t["sc"])
            nc.vector.tensor_mul(out=p, in0=va, in1=vb)
            t["vc"], t["p"] = vc, p

        def ncc_act(t):
            q = nccb.tile([128, NW], bf16, tag="q")
            r = nccb.tile([128, NW], bf16, tag="r")
            nc.scalar.activation(out=q, in_=t["p"], func=Act.Ln)
            nc.scalar.activation(out=r, in_=q, func=Act.Exp, scale=-0.5)
            t["r"] = r

        def ncc_fin(t, gi):
            fin = nccb.tile([128, NW], bf16, tag="fin")
            nc.vector.scalar_tensor_tensor(
                out=fin, in0=t["vc"], scalar=0.0, in1=t["r"],
                op0=Alu.add, op1=Alu.mult,
                accum_out=partsT[:, gi:gi + 1])

        # ---- batch 0 groups: pass B + NCC-pre, then variance terms
        t0 = []
        for g in range(NG):
            t0.append(pass_b_pre(0, g))
        for t in t0:
            ncc_mid(t)

        # ---- batch 1 pass A + drains (split DVE/ACT) while b0 NCC runs
        for v in VOLS:
            pass_a_vol(1, v, s1[v], split_drains=True)

        # ---- batch 0 NCC tail
        for t in t0:
            ncc_act(t)
        for g, t in enumerate(t0):
            ncc_fin(t, g)

        # ---- batch 1 groups
        t1 = []
        for g in range(NG):
            t1.append(pass_b_pre(1, g))
        for t in t1:
            ncc_mid(t)
        for t in t1:
            ncc_act(t)
        for g, t in enumerate(t1):
            ncc_fin(t, NG + g)

        nc.sync.dma_start(out=out_ext[:, :], in_=partsT)

    return nc


def _get_nc(split=True):
    if "nc" not in _CACHE:
        _CACHE["nc"] = _build_nc()
    if split and not _CACHE.get("split"):
        _split_multiwaits(_CACHE["nc"])
        _CACHE["split"] = True
    return _CACHE["nc"]


def _shards(y_true, y_pred):
    import ml_dtypes

    yt = np.ascontiguousarray(
        np.asarray(y_true, dtype=np.float32).reshape(B, D, H, W))
    yp = np.ascontiguousarray(
        np.asarray(y_pred, dtype=np.float32).reshape(B, D, H, W))
    pt = np.zeros((B, D + 2 * PAD, H, W), dtype=ml_dtypes.bfloat16)
    pp = np.zeros((B, D + 2 * PAD, H, W), dtype=ml_dtypes.bfloat16)
    pt[:, PAD:PAD + D] = yt.astype(ml_dtypes.bfloat16)
    pp[:, PAD:PAD + D] = yp.astype(ml_dtypes.bfloat16)

    BB = np.zeros((H, H), dtype=np.float32)
    for i in range(H):
        BB[i, max(0, i - PAD):min(H, i + PAD + 1)] = 1.0
    BB_bf16 = BB.astype(ml_dtypes.bfloat16)

    in_maps = []
    for c in range(NCORES):
        lo = c * D_OUT
        # transpose to [B, H, D_IN, W] so the on-device layout (partition=H)
        # is a fully contiguous DMA
        icore = np.ascontiguousarray(
            pt[:, lo:lo + D_IN].transpose(0, 2, 1, 3)).reshape(B, H, -1)
        jcore = np.ascontiguousarray(
            pp[:, lo:lo + D_IN].transpose(0, 2, 1, 3)).reshape(B, H, -1)
        in_maps.append({"I": icore, "J": jcore, "BB": BB_bf16})
    return in_maps


def run(y_true, y_pred, trace=False):
    from concourse.bass_utils import run_bass_kernel_spmd

    nc = _get_nc()
    in_maps = _shards(y_true, y_pred)
    res = run_bass_kernel_spmd(nc, in_maps, list(range(NCORES)), trace=trace)
    total = 0.0
    for r in res.results:
        total += float(np.asarray(r["partials"], dtype=np.float64).sum())
    loss = np.float32(1.0 - total / N_TOTAL)
    return np.array(loss, dtype=np.float32), res


def kernel(y_true, y_pred):
    loss, _ = run(y_true, y_pred, trace=False)
    return loss
